# revision 3
# baseline (speedup 1.0000x reference)
"""GQA prefill attention (B=1, T=2048, DIM=4096, 32 q-heads / 8 kv-heads),
tensor-parallel over 8 NeuronCores.

Sharding: core c owns kv head c and its 4 query heads: wq rows
[512c, 512c+512), wk/wv rows [128c, 128c+128), wo cols [512c, 512c+512).
Each core computes a partial y = attn_c @ wo_c.T in [T, DIM]; the host sums
the 8 partials (the "all-reduce after wo").

v2 design vs the 461us baseline:
  - All matmul operands bf16 (bf16 cadence 216ns/512col vs fp32r 227ns on
    this part, and no penalty for <256-col tiles).
  - Single software pipeline over t-chunks j: proj(j) -> attention(j, all
    4 heads) -> out-proj(j). Removes the phase-boundary stalls and spreads
    the y DMA across the whole run instead of a tail drain.
  - Softmax denominator: phat tiles are accumulated elementwise on the
    DVE/GpSimd engines (alternating) into pacc; ONE ones-matmul per
    (head, chunk) computes the cross-partition sum. This replaces one
    ones-matmul per s-tile (saves ~74k PE cycles).
  - Causal partial width: diagonal-crossing s-tiles only compute the
    unmasked 512-128*r columns of S/exp/select/add/O-accum.
  - Head finish chains (l-matmul -> recip -> PE broadcast -> normalize)
    are emitted one head late so the PE never waits on the DVE add chain.
"""

import sys

sys.path.insert(0, "/opt/trn_rl_repo")

import ml_dtypes
import numpy as np

import concourse.bass as bass
import concourse.tile as tile
from concourse import bacc, mybir
from concourse.bass_utils import run_bass_kernel_spmd
from concourse.masks import make_identity

T = 2048
DIM = 4096
HD = 128
NCORE = 8
NH = 4  # q heads per core
TCH = 512
NTCH = T // TCH  # 4 t-chunks
NST = T // 128  # 16 s-tiles
NDT = DIM // 128  # 32 d-tiles
F32 = mybir.dt.float32
F32R = mybir.dt.float32r
BF16 = mybir.dt.bfloat16
SCALE = 1.0 / float(np.sqrt(HD))

# test.py can flip these before calling kernel() to get profiling info
TRACE = False
LAST = {}

_CACHE = {}


def _build():
    nc = bacc.Bacc("TRN2", target_bir_lowering=False, debug=False, num_devices=NCORE)
    xT = nc.dram_tensor("xT", [DIM, T], BF16, kind="ExternalInput").ap()
    wqT = nc.dram_tensor("wqT", [DIM, NH * HD], BF16, kind="ExternalInput").ap()
    wkT = nc.dram_tensor("wkT", [DIM, HD], BF16, kind="ExternalInput").ap()
    wvT = nc.dram_tensor("wvT", [DIM, HD], BF16, kind="ExternalInput").ap()
    woT = nc.dram_tensor("woT", [NH * HD, DIM], BF16, kind="ExternalInput").ap()
    ones_in = nc.dram_tensor("ones", [128, 1], F32, kind="ExternalInput").ap()
    onescol_in = nc.dram_tensor("onescol", [1, 128], BF16, kind="ExternalInput").ap()
    y = nc.dram_tensor("y", [T, DIM], BF16, kind="ExternalOutput").ap()

    wqr = wqT.rearrange("(db p) f -> p db f", p=128)
    wkr = wkT.rearrange("(db p) f -> p db f", p=128)
    wvr = wvT.rearrange("(db p) f -> p db f", p=128)
    wor = woT.rearrange("(hb p) f -> p hb f", p=128)

    with tile.TileContext(nc) as tc:
        with (
            tc.tile_pool(name="persist", bufs=1) as persist,
            tc.tile_pool(name="xs", bufs=6) as xs,
            tc.tile_pool(name="phs", bufs=6) as phs,
            tc.tile_pool(name="paccs", bufs=2) as paccs,
            tc.tile_pool(name="vts", bufs=2) as vts,
            tc.tile_pool(name="recips", bufs=2) as recips,
            tc.tile_pool(name="recipbs", bufs=2) as recipbs,
            tc.tile_pool(name="rbcs", bufs=2) as rbcs,
            tc.tile_pool(name="ys", bufs=6) as ys,
        ):
            qt_sb = [persist.tile([128, T], BF16, tag=f"qt{h}", name=f"qt{h}") for h in range(NH)]
            kt_sb = persist.tile([128, T], BF16, tag="kt")
            v_sb = persist.tile([128, NST, HD], BF16, tag="v")
            ao_sb = [persist.tile([128, TCH], BF16, tag=f"ao{h}", name=f"ao{h}") for h in range(NH)]
            wq_sb = persist.tile([128, NDT, NH * HD], BF16, tag="wq")
            wk_sb = persist.tile([128, NDT, HD], BF16, tag="wk")
            wv_sb = persist.tile([128, NDT, HD], BF16, tag="wv")
            wo_sb = persist.tile([128, NH, DIM], BF16, tag="wo")
            ones_sb = persist.tile([128, 1], F32R, tag="ones")
            onescol = persist.tile([1, 128], BF16, tag="onescol")
            ident = persist.tile([128, 128], BF16, tag="ident")
            nc.sync.dma_start(out=ones_sb, in_=ones_in.bitcast(F32R))
            nc.sync.dma_start(out=onescol, in_=onescol_in)
            make_identity(nc, ident)
            # warm the exp activation table before first real use
            expwarm = persist.tile([1, 2], F32, tag="expwarm")
            nc.vector.memset(expwarm, 0.0)
            nc.scalar.activation(
                out=expwarm[:],
                in_=expwarm[:],
                func=mybir.ActivationFunctionType.Exp,
                scale=1.0,
            )

            # state for the deferred head-finish chains
            pend = {}  # h -> (pacc, psum_ot)

            def finish_head_l(h, lp):
                """Emit the l-matmul for head h (PE)."""
                pacc, _ = pend[h]
                psum_l = lp.tile([1, TCH], F32, tag="l", name=f"l{h}")
                nc.tensor.matmul(psum_l[:], ones_sb[:], pacc[:], start=True, stop=True)
                return psum_l

            def finish_head_rest(h, psum_l, prbp):
                """recip -> PE broadcast -> normalize into ao_sb[h]."""
                _, psum_ot = pend.pop(h)
                recip = recips.tile([1, TCH], F32, tag="recip")
                nc.vector.reciprocal_approx_fast(recip[:], psum_l[:])
                recip_bf = recipbs.tile([1, TCH], BF16, tag="recipb")
                nc.vector.tensor_copy(recip_bf[:], recip[:])
                prb = prbp.tile([128, TCH], F32, tag="prb", name=f"prb{h}")
                nc.tensor.matmul(prb[:], onescol[:], recip_bf[:], start=True, stop=True)
                rbc = rbcs.tile([128, TCH], F32, tag="rbc")
                nc.scalar.copy(rbc[:], prb[:])
                nc.vector.tensor_mul(ao_sb[h][:], psum_ot[:], rbc[:])

            for j in range(NTCH):
                cs = slice(j * TCH, (j + 1) * TCH)
                # ---------------- projections for chunk j ----------------
                with (
                    tc.tile_pool(name=f"pj{j}", bufs=1, space="PSUM") as psp,
                    tc.tile_pool(name=f"tr{j}", bufs=1, space="PSUM") as ptr,
                ):
                    qps = [
                        psp.tile([128, TCH], F32, tag=f"projq{fq}", name=f"projq{fq}")
                        for fq in range(NH)
                    ]
                    kps = psp.tile([128, TCH], F32, tag="projk")
                    vps = psp.tile([128, TCH], F32, tag="projv")
                    for d in range(NDT):
                        if j == 0 and (d in (0, 1) or (d >= 4 and d % 4 == 0)):
                            # first two tiny groups so the PE starts almost
                            # immediately; bigger groups once streaming
                            g = slice(d, d + (1 if d == 0 else 3 if d == 1 else 4))
                            nc.scalar.dma_start(out=wq_sb[:, g, :], in_=wqr[:, g, :])
                            nc.scalar.dma_start(out=wk_sb[:, g, :], in_=wkr[:, g, :])
                            nc.scalar.dma_start(out=wv_sb[:, g, :], in_=wvr[:, g, :])
                        xbf = xs.tile([128, TCH], BF16, tag="xbf")
                        nc.sync.dma_start(out=xbf, in_=xT[d * 128 : (d + 1) * 128, cs])
                        st = d == 0
                        sp = d == NDT - 1
                        for fq in range(NH):
                            nc.tensor.matmul(
                                qps[fq][:],
                                wq_sb[:, d, fq * HD : (fq + 1) * HD],
                                xbf[:],
                                start=st,
                                stop=sp,
                            )
                        nc.tensor.matmul(kps[:], wk_sb[:, d, :], xbf[:], start=st, stop=sp)
                        nc.tensor.matmul(vps[:], wv_sb[:, d, :], xbf[:], start=st, stop=sp)
                    if j == 0:
                        # wo arrives during attention(0); needed at out-proj(0)
                        for hb in range(NH):
                            nc.scalar.dma_start(out=wo_sb[:, hb, :], in_=wor[:, hb, :])
                    for fq in range(NH):
                        nc.scalar.copy(qt_sb[fq][:, cs], qps[fq][:])
                    nc.vector.tensor_copy(kt_sb[:, cs], kps[:])
                    vt_tmp = vts.tile([128, TCH], BF16, tag="vt")
                    nc.vector.tensor_copy(vt_tmp[:], vps[:])
                    for ii in range(4):
                        ptrt = ptr.tile([128, HD], BF16, tag="tr")
                        nc.tensor.transpose(
                            ptrt[:], vt_tmp[:, ii * 128 : (ii + 1) * 128], ident[:]
                        )
                        nc.vector.tensor_copy(v_sb[:, 4 * j + ii, :], ptrt[:])

                # ---------------- attention + out-proj for chunk j ----------------
                n_i = 4 * j + 4
                with (
                    tc.tile_pool(name=f"st{j}", bufs=2, space="PSUM") as stp,
                    tc.tile_pool(name=f"ot{j}", bufs=2, space="PSUM") as otp,
                    tc.tile_pool(name=f"lp{j}", bufs=1, space="PSUM") as lp,
                    tc.tile_pool(name=f"prb{j}", bufs=1, space="PSUM") as prbp,
                ):
                    for h in range(NH):
                        pacc = paccs.tile([128, TCH], F32R, tag="pacc", name=f"pacc{h}")
                        psum_ot = otp.tile([128, TCH], F32, tag="ot", name=f"ot{h}")
                        pend[h] = (pacc, psum_ot)
                        for i in range(n_i):
                            r = i - 4 * j
                            c0 = 128 * r if r > 0 else 0  # masked col prefix
                            psum_st = stp.tile([128, TCH], F32, tag="st")
                            nc.tensor.matmul(
                                psum_st[:, c0:],
                                kt_sb[:, i * 128 : (i + 1) * 128],
                                qt_sb[h][:, j * TCH + c0 : (j + 1) * TCH],
                                start=True,
                                stop=True,
                            )
                            phat = phs.tile([128, TCH], BF16, tag="phat")
                            nc.scalar.activation(
                                out=phat[:, c0:],
                                in_=psum_st[:, c0:],
                                func=mybir.ActivationFunctionType.Exp,
                                scale=SCALE,
                            )
                            if r >= 0:  # diagonal tile: zero where s > t
                                nc.gpsimd.affine_select(
                                    out=phat[:, c0:],
                                    in_=phat[:, c0:],
                                    compare_op=mybir.AluOpType.is_ge,
                                    fill=0.0,
                                    base=0,
                                    pattern=[[1, TCH - c0]],
                                    channel_multiplier=-1,
                                )
                            nc.tensor.matmul(
                                psum_ot[:, c0:],
                                v_sb[:, i, :],
                                phat[:, c0:],
                                start=(i == 0),
                                stop=(i == n_i - 1),
                            )
                            if i == 0:
                                nc.vector.tensor_copy(pacc[:], phat[:])
                            else:
                                eng = nc.vector if (i % 2 == 0) else nc.gpsimd
                                eng.tensor_add(
                                    pacc[:, c0:], pacc[:, c0:], phat[:, c0:]
                                )
                        if h > 0:
                            psum_l = finish_head_l(h - 1, lp)
                            finish_head_rest(h - 1, psum_l, prbp)

                    # out-proj for chunk j, interleaved with head 3's finish
                    with tc.tile_pool(name=f"psy{j}", bufs=2, space="PSUM") as psy:
                        psum_l3 = finish_head_l(3, lp)
                        pys = {}
                        for fc in range(2):
                            pys[fc] = psy.tile(
                                [128, 512], F32, tag="py", name=f"py{fc}"
                            )
                            for hb in range(3):
                                nc.tensor.matmul(
                                    pys[fc][:],
                                    ao_sb[hb][:, 0:128],
                                    wo_sb[:, hb, fc * 512 : (fc + 1) * 512],
                                    start=(hb == 0),
                                    stop=False,
                                )
                        finish_head_rest(3, psum_l3, prbp)
                        for tt4 in range(4):
                            tloc = slice(tt4 * 128, (tt4 + 1) * 128)
                            tsl = slice(j * TCH + tt4 * 128, j * TCH + (tt4 + 1) * 128)
                            for fc in range(8):
                                fsl = slice(fc * 512, (fc + 1) * 512)
                                if tt4 == 0 and fc < 2:
                                    py = pys.pop(fc)
                                    hbs = [3]  # 0..2 already accumulated above
                                else:
                                    py = psy.tile([128, 512], F32, tag="py")
                                    hbs = [0, 1, 2, 3]
                                for hb in hbs:
                                    nc.tensor.matmul(
                                        py[:],
                                        ao_sb[hb][:, tloc],
                                        wo_sb[:, hb, fsl],
                                        start=(hb == 0),
                                        stop=(hb == 3),
                                    )
                                yt = ys.tile([128, 512], BF16, tag="yt")
                                if fc % 2 == 0:
                                    nc.vector.tensor_copy(yt[:], py[:])
                                    nc.sync.dma_start(out=y[tsl, fsl], in_=yt[:])
                                else:
                                    nc.scalar.copy(yt[:], py[:])
                                    nc.scalar.dma_start(out=y[tsl, fsl], in_=yt[:])

    nc.compile()
    return nc


def kernel(x, wq, wk, wv, wo):
    x = np.asarray(x, dtype=np.float32)
    wq = np.asarray(wq, dtype=np.float32)
    wk = np.asarray(wk, dtype=np.float32)
    wv = np.asarray(wv, dtype=np.float32)
    wo = np.asarray(wo, dtype=np.float32)

    if "nc" not in _CACHE:
        _CACHE["nc"] = _build()
    nc = _CACHE["nc"]

    xT = np.ascontiguousarray(x[0].T).astype(ml_dtypes.bfloat16)  # [DIM, T]
    ones = np.ones((128, 1), np.float32)
    onescol = np.ones((1, 128), ml_dtypes.bfloat16)
    in_maps = []
    for c in range(NCORE):
        qs = slice(c * NH * HD, (c + 1) * NH * HD)
        ks = slice(c * HD, (c + 1) * HD)
        in_maps.append(
            {
                "xT": xT,
                "wqT": np.ascontiguousarray(wq[qs, :].T).astype(ml_dtypes.bfloat16),
                "wkT": np.ascontiguousarray(wk[ks, :].T).astype(ml_dtypes.bfloat16),
                "wvT": np.ascontiguousarray(wv[ks, :].T).astype(ml_dtypes.bfloat16),
                "woT": np.ascontiguousarray(wo[:, qs].T).astype(ml_dtypes.bfloat16),
                "ones": ones,
                "onescol": onescol,
            }
        )

    res = run_bass_kernel_spmd(
        nc, in_maps, core_ids=list(range(NCORE)), trace=TRACE
    )
    LAST["results"] = res

    out = np.zeros((T, DIM), dtype=np.float64)
    for c in range(NCORE):
        out += res.results[c]["y"].astype(np.float64)
    return out.astype(np.float32).reshape(1, T, DIM)


# revision 11
# speedup vs baseline: 1.0198x; 1.0198x over previous
"""GQA prefill attention (B=1, T=2048, DIM=4096, 32 q-heads / 8 kv-heads),
tensor-parallel over 8 NeuronCores.

Sharding: core c owns kv head c and its 4 query heads: wq rows
[512c, 512c+512), wk/wv rows [128c, 128c+128), wo cols [512c, 512c+512).
Each core computes a partial y = attn_c @ wo_c.T in [T, DIM]; the host sums
the 8 partials (the "all-reduce after wo").

v3 design:
  - All matmul operands bf16; PSUM accumulation f32.
  - Single software pipeline over t-chunks j: proj(j) -> attention(j, all
    4 heads) -> out-proj(j).
  - Causal masking by matmul: for diagonal-crossing tiles, one extra
    N=128 matmul accumulates -C*relu(s-t) into the S psum (A[m,s]=[s>=m],
    B[m,t]=-C*[m>t], so (A^T B)[s,t] = -C*(s-t) for s>t), and exp then
    underflows to exactly 0 in the masked region. No gpsimd affine_select
    on the critical path.
  - Softmax denominator: phat tiles accumulate elementwise into two
    engine-local accumulators (DVE: even i, GpSimd: odd i); two
    accumulating ones-matmuls per (head, chunk) give the denominator.
  - Causal partial width: diagonal tiles only compute the 512-128*r
    unmasked columns of S/exp/add/O-accum.
  - Head finish chains (l-matmul -> recip -> PE broadcast -> normalize)
    are emitted one head late so the PE never waits on the add chains.
"""

import sys

sys.path.insert(0, "/opt/trn_rl_repo")

import ml_dtypes
import numpy as np

import concourse.bass as bass
import concourse.tile as tile
from concourse import bacc, mybir
from concourse.bass_utils import run_bass_kernel_spmd
from concourse.masks import make_identity

T = 2048
DIM = 4096
HD = 128
NCORE = 8
NH = 4  # q heads per core
TCH = 512
NTCH = T // TCH  # 4 t-chunks
NST = T // 128  # 16 s-tiles
NDT = DIM // 128  # 32 d-tiles
F32 = mybir.dt.float32
F32R = mybir.dt.float32r
BF16 = mybir.dt.bfloat16
SCALE = 1.0 / float(np.sqrt(HD))
MASKC = 8192.0  # big, bf16-exact; C*1 dwarfs any |score| here

# test.py can flip these before calling kernel() to get profiling info
TRACE = False
LAST = {}

_CACHE = {}


def _build():
    nc = bacc.Bacc("TRN2", target_bir_lowering=False, debug=False, num_devices=NCORE)
    xT = nc.dram_tensor("xT", [DIM, T], BF16, kind="ExternalInput").ap()
    wqT = nc.dram_tensor("wqT", [DIM, NH * HD], BF16, kind="ExternalInput").ap()
    wkT = nc.dram_tensor("wkT", [DIM, HD], BF16, kind="ExternalInput").ap()
    wvT = nc.dram_tensor("wvT", [DIM, HD], BF16, kind="ExternalInput").ap()
    woT = nc.dram_tensor("woT", [NH * HD, DIM], BF16, kind="ExternalInput").ap()
    ones_in = nc.dram_tensor("ones", [128, 1], F32, kind="ExternalInput").ap()
    onescol_in = nc.dram_tensor("onescol", [1, 128], BF16, kind="ExternalInput").ap()
    maskA_in = nc.dram_tensor("maskA", [128, 128], BF16, kind="ExternalInput").ap()
    maskB_in = nc.dram_tensor("maskB", [128, 128], BF16, kind="ExternalInput").ap()
    y = nc.dram_tensor("y", [T, DIM], BF16, kind="ExternalOutput").ap()

    wqr = wqT.rearrange("(db p) f -> p db f", p=128)
    wkr = wkT.rearrange("(db p) f -> p db f", p=128)
    wvr = wvT.rearrange("(db p) f -> p db f", p=128)
    wor = woT.rearrange("(hb p) f -> p hb f", p=128)

    with tile.TileContext(nc) as tc:
        with (
            tc.tile_pool(name="persist", bufs=1) as persist,
            tc.tile_pool(name="xs", bufs=6) as xs,
            tc.tile_pool(name="phs", bufs=6) as phs,
            tc.tile_pool(name="paccs", bufs=2) as paccs,
            tc.tile_pool(name="vts", bufs=2) as vts,
            tc.tile_pool(name="recips", bufs=2) as recips,
            tc.tile_pool(name="recipbs", bufs=2) as recipbs,
            tc.tile_pool(name="rbcs", bufs=2) as rbcs,
            tc.tile_pool(name="ys", bufs=6) as ys,
        ):
            qt_sb = [persist.tile([128, T], BF16, tag=f"qt{h}", name=f"qt{h}") for h in range(NH)]
            kt_sb = persist.tile([128, T], BF16, tag="kt")
            v_sb = persist.tile([128, NST, HD], BF16, tag="v")
            ao_sb = [persist.tile([128, TCH], BF16, tag=f"ao{h}", name=f"ao{h}") for h in range(NH)]
            wq_sb = persist.tile([128, NDT, NH * HD], BF16, tag="wq")
            wk_sb = persist.tile([128, NDT, HD], BF16, tag="wk")
            wv_sb = persist.tile([128, NDT, HD], BF16, tag="wv")
            wo_sb = persist.tile([128, NH, DIM], BF16, tag="wo")
            ones_sb = persist.tile([128, 1], F32R, tag="ones")
            onescol = persist.tile([1, 128], BF16, tag="onescol")
            maskA = persist.tile([128, 128], BF16, tag="maskA")
            maskB = persist.tile([128, 128], BF16, tag="maskB")
            ident = persist.tile([128, 128], BF16, tag="ident")
            nc.sync.dma_start(out=ones_sb, in_=ones_in.bitcast(F32R))
            nc.sync.dma_start(out=onescol, in_=onescol_in)
            nc.sync.dma_start(out=maskA, in_=maskA_in)
            nc.sync.dma_start(out=maskB, in_=maskB_in)
            make_identity(nc, ident)
            # warm the exp activation table before first real use
            expwarm = persist.tile([1, 2], F32, tag="expwarm")
            nc.vector.memset(expwarm, 0.0)
            nc.scalar.activation(
                out=expwarm[:],
                in_=expwarm[:],
                func=mybir.ActivationFunctionType.Exp,
                scale=1.0,
            )

            # state for the deferred head-finish chains
            pend = {}  # h -> (pacc_v, pacc_g, psum_ot)

            def finish_head_l(h, lp):
                """Emit the denominator matmuls for head h (PE)."""
                pacc_v, pacc_g, _ = pend[h]
                psum_l = lp.tile([1, TCH], F32, tag="l", name=f"l{h}")
                if pacc_g is None:
                    nc.tensor.matmul(psum_l[:], ones_sb[:], pacc_v[:], start=True, stop=True)
                else:
                    nc.tensor.matmul(psum_l[:], ones_sb[:], pacc_v[:], start=True, stop=False)
                    nc.tensor.matmul(psum_l[:], ones_sb[:], pacc_g[:], start=False, stop=True)
                return psum_l

            def finish_head_rest(h, psum_l, prbp):
                """recip -> PE broadcast -> normalize into ao_sb[h]."""
                _, _, psum_ot = pend.pop(h)
                recip = recips.tile([1, TCH], F32, tag="recip")
                nc.vector.reciprocal_approx_fast(recip[:], psum_l[:])
                recip_bf = recipbs.tile([1, TCH], BF16, tag="recipb")
                nc.vector.tensor_copy(recip_bf[:], recip[:])
                prb = prbp.tile([128, TCH], F32, tag="prb", name=f"prb{h}")
                nc.tensor.matmul(prb[:], onescol[:], recip_bf[:], start=True, stop=True)
                rbc = rbcs.tile([128, TCH], F32, tag="rbc")
                nc.scalar.copy(rbc[:], prb[:])
                nc.vector.tensor_mul(ao_sb[h][:], psum_ot[:], rbc[:])

            for j in range(NTCH):
                cs = slice(j * TCH, (j + 1) * TCH)
                # ---------------- projections for chunk j ----------------
                with (
                    tc.tile_pool(name=f"pj{j}", bufs=1, space="PSUM") as psp,
                    tc.tile_pool(name=f"tr{j}", bufs=1, space="PSUM") as ptr,
                ):
                    qps = [
                        psp.tile([128, TCH], F32, tag=f"projq{fq}", name=f"projq{fq}")
                        for fq in range(NH)
                    ]
                    kps = psp.tile([128, TCH], F32, tag="projk")
                    vps = psp.tile([128, TCH], F32, tag="projv")
                    for d in range(NDT):
                        if j == 0 and (d in (0, 1) or (d >= 4 and d % 4 == 0)):
                            # first two tiny groups so the PE starts almost
                            # immediately; bigger groups once streaming
                            g = slice(d, d + (1 if d == 0 else 3 if d == 1 else 4))
                            nc.scalar.dma_start(out=wq_sb[:, g, :], in_=wqr[:, g, :])
                            nc.scalar.dma_start(out=wk_sb[:, g, :], in_=wkr[:, g, :])
                            nc.scalar.dma_start(out=wv_sb[:, g, :], in_=wvr[:, g, :])
                        xbf = xs.tile([128, TCH], BF16, tag="xbf")
                        nc.sync.dma_start(out=xbf, in_=xT[d * 128 : (d + 1) * 128, cs])
                        st = d == 0
                        sp = d == NDT - 1
                        for fq in range(NH):
                            nc.tensor.matmul(
                                qps[fq][:],
                                wq_sb[:, d, fq * HD : (fq + 1) * HD],
                                xbf[:],
                                start=st,
                                stop=sp,
                            )
                        nc.tensor.matmul(kps[:], wk_sb[:, d, :], xbf[:], start=st, stop=sp)
                        nc.tensor.matmul(vps[:], wv_sb[:, d, :], xbf[:], start=st, stop=sp)
                    if j == 0:
                        # wo arrives during attention(0); needed at out-proj(0).
                        # On the gpsimd queue: scalar queue still streams wq/wk/wv.
                        for hb in range(NH):
                            nc.gpsimd.dma_start(out=wo_sb[:, hb, :], in_=wor[:, hb, :])
                    for fq in range(NH):
                        if fq % 2 == 0:
                            nc.scalar.copy(qt_sb[fq][:, cs], qps[fq][:])
                        else:
                            nc.vector.tensor_copy(qt_sb[fq][:, cs], qps[fq][:])
                    nc.vector.tensor_copy(kt_sb[:, cs], kps[:])
                    vt_tmp = vts.tile([128, TCH], BF16, tag="vt")
                    nc.vector.tensor_copy(vt_tmp[:], vps[:])
                    for ii in range(4):
                        ptrt = ptr.tile([128, HD], BF16, tag="tr")
                        nc.tensor.transpose(
                            ptrt[:], vt_tmp[:, ii * 128 : (ii + 1) * 128], ident[:]
                        )
                        nc.vector.tensor_copy(v_sb[:, 4 * j + ii, :], ptrt[:])

                # ---------------- attention + out-proj for chunk j ----------------
                n_i = 4 * j + 4
                with (
                    tc.tile_pool(name=f"ot{j}", bufs=3, space="PSUM") as otp,
                    tc.tile_pool(name=f"lp{j}", bufs=1, space="PSUM") as lp,
                    tc.tile_pool(name=f"prb{j}", bufs=1, space="PSUM") as prbp,
                ):
                    with tc.tile_pool(name=f"st{j}", bufs=3, space="PSUM") as stp:
                        for h in range(NH):
                            pacc_v = paccs.tile([128, TCH], F32R, tag="paccv", name=f"paccv{h}")
                            # j=0 only has 4 s-tiles and tiles 1+ are partial
                            # width, so a second accumulator can't be seeded
                            # with a full-width copy there; use DVE only.
                            pacc_g = (
                                paccs.tile([128, TCH], F32R, tag="paccg", name=f"paccg{h}")
                                if j > 0
                                else None
                            )
                            psum_ot = otp.tile([128, TCH], F32, tag="ot", name=f"ot{h}")
                            pend[h] = (pacc_v, pacc_g, psum_ot)
                            for i in range(n_i):
                                r = i - 4 * j
                                c0 = 128 * r if r > 0 else 0  # masked col prefix
                                psum_st = stp.tile([128, TCH], F32, tag="st")
                                nc.tensor.matmul(
                                    psum_st[:, c0:],
                                    kt_sb[:, i * 128 : (i + 1) * 128],
                                    qt_sb[h][:, j * TCH + c0 : (j + 1) * TCH],
                                    start=True,
                                    stop=True,
                                )
                                if r >= 0:  # diagonal tile: psum += -C*relu(s-t)
                                    nc.tensor.matmul(
                                        psum_st[:, c0 : c0 + 128],
                                        maskA[:],
                                        maskB[:],
                                        start=False,
                                        stop=True,
                                        skip_group_check=True,
                                    )
                                phat = phs.tile([128, TCH], BF16, tag="phat")
                                nc.scalar.activation(
                                    out=phat[:, c0:],
                                    in_=psum_st[:, c0:],
                                    func=mybir.ActivationFunctionType.Exp,
                                    scale=SCALE,
                                )
                                nc.tensor.matmul(
                                    psum_ot[:, c0:],
                                    v_sb[:, i, :],
                                    phat[:, c0:],
                                    start=(i == 0),
                                    stop=(i == n_i - 1),
                                )
                                if pacc_g is None:
                                    eng, pacc, first = nc.vector, pacc_v, i == 0
                                else:
                                    eng = nc.vector if (i % 2 == 0) else nc.gpsimd
                                    pacc = pacc_v if (i % 2 == 0) else pacc_g
                                    first = i < 2
                                if first:
                                    assert c0 == 0, "first tile must be full width"
                                    eng.tensor_copy(pacc[:], phat[:])
                                else:
                                    eng.tensor_add(pacc[:, c0:], pacc[:, c0:], phat[:, c0:])
                            if h > 0:
                                psum_l = finish_head_l(h - 1, lp)
                                finish_head_rest(h - 1, psum_l, prbp)

                    # out-proj for chunk j, interleaved with head 3's finish
                    with tc.tile_pool(name=f"psy{j}", bufs=2, space="PSUM") as psy:
                        psum_l3 = finish_head_l(3, lp)
                        pys = {}
                        for fc in range(2):
                            pys[fc] = psy.tile([128, 512], F32, tag="py", name=f"py{fc}")
                            for hb in range(3):
                                nc.tensor.matmul(
                                    pys[fc][:],
                                    ao_sb[hb][:, 0:128],
                                    wo_sb[:, hb, fc * 512 : (fc + 1) * 512],
                                    start=(hb == 0),
                                    stop=False,
                                )
                        finish_head_rest(3, psum_l3, prbp)
                        for tt4 in range(4):
                            tloc = slice(tt4 * 128, (tt4 + 1) * 128)
                            tsl = slice(j * TCH + tt4 * 128, j * TCH + (tt4 + 1) * 128)
                            for fc in range(8):
                                fsl = slice(fc * 512, (fc + 1) * 512)
                                if tt4 == 0 and fc < 2:
                                    py = pys.pop(fc)
                                    hbs = [3]  # 0..2 already accumulated above
                                else:
                                    py = psy.tile([128, 512], F32, tag="py")
                                    hbs = [0, 1, 2, 3]
                                for hb in hbs:
                                    nc.tensor.matmul(
                                        py[:],
                                        ao_sb[hb][:, tloc],
                                        wo_sb[:, hb, fsl],
                                        start=(hb == 0),
                                        stop=(hb == 3),
                                    )
                                yt = ys.tile([128, 512], BF16, tag="yt")
                                if fc % 2 == 0:
                                    nc.vector.tensor_copy(yt[:], py[:])
                                    nc.gpsimd.dma_start(out=y[tsl, fsl], in_=yt[:])
                                else:
                                    nc.scalar.copy(yt[:], py[:])
                                    nc.scalar.dma_start(out=y[tsl, fsl], in_=yt[:])

    nc.compile()
    return nc


def kernel(x, wq, wk, wv, wo):
    x = np.asarray(x, dtype=np.float32)
    wq = np.asarray(wq, dtype=np.float32)
    wk = np.asarray(wk, dtype=np.float32)
    wv = np.asarray(wv, dtype=np.float32)
    wo = np.asarray(wo, dtype=np.float32)

    if "nc" not in _CACHE:
        _CACHE["nc"] = _build()
    nc = _CACHE["nc"]

    xT = np.ascontiguousarray(x[0].T).astype(ml_dtypes.bfloat16)  # [DIM, T]
    ones = np.ones((128, 1), np.float32)
    onescol = np.ones((1, 128), ml_dtypes.bfloat16)
    idx = np.arange(128)
    maskA = (idx[None, :] >= idx[:, None]).astype(ml_dtypes.bfloat16)  # [m,s]: s>=m
    maskB = ((idx[:, None] > idx[None, :]) * -MASKC).astype(ml_dtypes.bfloat16)  # [m,t]
    in_maps = []
    for c in range(NCORE):
        qs = slice(c * NH * HD, (c + 1) * NH * HD)
        ks = slice(c * HD, (c + 1) * HD)
        in_maps.append(
            {
                "xT": xT,
                "wqT": np.ascontiguousarray(wq[qs, :].T).astype(ml_dtypes.bfloat16),
                "wkT": np.ascontiguousarray(wk[ks, :].T).astype(ml_dtypes.bfloat16),
                "wvT": np.ascontiguousarray(wv[ks, :].T).astype(ml_dtypes.bfloat16),
                "woT": np.ascontiguousarray(wo[:, qs].T).astype(ml_dtypes.bfloat16),
                "ones": ones,
                "onescol": onescol,
                "maskA": maskA,
                "maskB": maskB,
            }
        )

    res = run_bass_kernel_spmd(
        nc, in_maps, core_ids=list(range(NCORE)), trace=TRACE
    )
    LAST["results"] = res

    out = np.zeros((T, DIM), dtype=np.float64)
    for c in range(NCORE):
        out += res.results[c]["y"].astype(np.float64)
    return out.astype(np.float32).reshape(1, T, DIM)
